# revision 15
# baseline (speedup 1.0000x reference)
"""Multi-head GAT (N=6144, F_IN=64, F_OUT=32, H=4) on 8 TRN2 NeuronCores.

Sharding: each core owns a 768-row block of src nodes (N/8); the cheap
O(N*F) work (h_prime, dst scores) is recomputed redundantly per core from
the replicated h (cheaper than any collective), and the O(N^2) attention
work is sharded by src block. No collectives.

Score-stage math (one DVE op per [128 x 768] tile): softmax over dst m is
invariant to per-src-row positive rescaling, and
  exp(leaky_relu(s+d)) = max(e^{s+d}, e^{0.2(s+d)})
                       = e^{s} * e^{0.2 d} * max(e^{0.8 d}, e^{-0.8 s})
so the e^{s[n]} factor is dropped. With
  g[n] = exp(-0.8 s[n])  (broadcast tile per head, via PE rank-1 matmul + ACT)
  rr[m] = exp(0.8 d[m]), e02[m] = exp(0.2 d[m])  (per-partition scalars)
the rescaled unnormalized probability is
  P'[m,n] = (g[n] max rr[m]) * e02[m]   ... one fused tensor_scalar (bf16 4x).
The softmax denominator comes free from a ones column appended to h_prime in
the accumulated output matmul out.T[33, 768] (PSUM, 48 m-tiles); the final
division is a per-partition-scalar mult on ACT after a PE transpose.

PE optimizations:
  - h_prime+scores matmul uses a bf16 hi/lo split (h = hi + lo, w = whi + wlo;
    hi@whi + lo@whi + hi@wlo) -> ~fp32 accuracy at bf16 speed (fp32 matmul
    runs at 1/4 rate).
  - src-score broadcast matmuls run as float32r (full-rate fp32 mode).
  - the two heads of a pair run their output matmuls CONCURRENTLY via PE
    column tiling: head A at tile_position (0,0) (array cols 0-32), head B at
    (0,64) (cols 64-96), separate PSUM tiles so the start=True bank clear of
    one cannot wipe the other's accumulation.
"""
import numpy as np

try:
    import concourse.bass  # noqa: F401
except Exception:  # pragma: no cover
    import sys
    for p in ("/opt/trn_rl_repo", "/root/.axon_site/_ro/trn_rl_repo"):
        sys.path.insert(0, p)

import json as _json

import ml_dtypes
import concourse.bass as bass
import concourse.mybir as mybir
import concourse.tile as tile

N, F_IN, F_OUT, NH = 6144, 64, 32, 4
ACT_EVERY = 16               # every ACT_EVERY-th m-tile's P' comes from ACT
ACT_T0 = 3                   # first ACT tile (must be > 0: bank-clear order)
NCORES = 8
BLK = N // NCORES            # 768 src rows per core
NT = N // 128                # 48 m-tiles
PDT = mybir.dt.bfloat16      # dtype for P' / h_prime / g tiles
F32 = mybir.dt.float32
BF16 = mybir.dt.bfloat16
F32R = mybir.dt.float32r

_cache = {}


def _split_waits(bir_json: bytes, limit: int = 1) -> bytes:
    # This walrus build accepts at most ONE sync-wait per instruction
    # ("Too many sync wait commands" otherwise). Tile emits several; split
    # the excess onto NoOp carriers placed just before, on the same engine.
    m = _json.loads(bir_json)
    ctr = 0
    for fn in m["functions"]:
        for bb in fn["blocks"]:
            new_insts = []
            for ins in bb["instructions"]:
                si = ins.get("sync_info")
                waits = (si or {}).get("on_wait") or []
                if len(waits) > limit:
                    excess, keep = waits[:-limit], waits[-limit:]
                    for i in range(0, len(excess), limit):
                        ctr += 1
                        new_insts.append({
                            "debug": ins.get("debug", 0),
                            "engine": ins["engine"],
                            "ins": [], "outs": [],
                            "name": f"WSPLIT-{ctr}",
                            "opcode": "NoOp",
                            "sync_info": {"on_wait": excess[i:i + limit],
                                          "on_update": []},
                        })
                    si["on_wait"] = keep
                new_insts.append(ins)
            bb["instructions"] = new_insts
    return _json.dumps(m).encode()


def _build_program(outer_reps: int = 1):
    AF = mybir.ActivationFunctionType
    OP = mybir.AluOpType
    nc = bass.Bass()

    hthi_d = nc.dram_tensor("hthi", [F_IN, N], BF16, kind="ExternalInput")
    htlo_d = nc.dram_tensor("htlo", [F_IN, N], BF16, kind="ExternalInput")
    # packed small operands: [hbhi 768 | hblo 768 | wvhi 132 | wvlo 132 |
    #                          vrhi 512 | vrlo 512]  (bf16, one DMA)
    PK = 2 * BLK + 2 * 132 + 2 * 512
    pk_d = nc.dram_tensor("pack", [F_IN, PK], BF16, kind="ExternalInput")
    id_d = nc.dram_tensor("identb", [33, 33], F32, kind="ExternalInput")
    out_d = nc.dram_tensor("out", [BLK, NH * F_OUT], F32, kind="ExternalOutput")

    with tile.TileContext(nc) as tc:
        with (
            tc.tile_pool(name="singles", bufs=1) as singles,
            tc.tile_pool(name="ppool", bufs=6) as ppool,
            tc.tile_pool(name="otp", bufs=2) as otp,
            tc.tile_pool(name="rcp", bufs=4) as rcp,
            tc.tile_pool(name="psA", bufs=2, space="PSUM") as psA,
            tc.tile_pool(name="psB", bufs=1, space="PSUM") as psB,
            tc.tile_pool(name="psC", bufs=2, space="PSUM") as psC,
        ):
            hthi_sb = singles.tile([F_IN, N], BF16)
            htlo_sb = singles.tile([F_IN, N], BF16)
            pk_sb = singles.tile([F_IN, PK], BF16)
            hbh_sb = pk_sb[:, 0:BLK]
            hbl_sb = pk_sb[:, BLK:2 * BLK]
            wvh_sb = pk_sb[:, 2 * BLK:2 * BLK + 132]
            wvl_sb = pk_sb[:, 2 * BLK + 132:2 * BLK + 264]
            vrh_sb = pk_sb[:, 2 * BLK + 264:2 * BLK + 776]
            vrl_sb = pk_sb[:, 2 * BLK + 776:2 * BLK + 1288]
            id_sb = singles.tile([33, 33], F32)
            hp_sb = singles.tile([128, NT * NH * 33], PDT)
            gb_sb = singles.tile([128, NH * BLK], PDT)
            rr_sb = singles.tile([128, NT * NH], F32)
            e02_sb = singles.tile([128, NT * NH], F32)
            mc_sb = singles.tile([128, NT * NH], F32)
            cb_sb = singles.tile([128, NT * NH], BF16)
            outsb = singles.tile([128, 6 * 128], F32)

            # ones columns of hp (denominator lane of the output matmul)
            hp_v = hp_sb[:].rearrange("p (g x) -> p g x", x=33)
            nc.gpsimd.memset(hp_v[:, :, 32:33], 1.0)

            for orep in range(outer_reps):
                nc.sync.dma_start(out=pk_sb[:], in_=pk_d[:])
                nc.sync.dma_start(out=id_sb[:], in_=id_d[:])
                for k in range(6):
                    sl = slice(k * 1024, (k + 1) * 1024)
                    nc.sync.dma_start(out=hthi_sb[:, sl], in_=hthi_d[:, sl])
                    nc.sync.dma_start(out=htlo_sb[:, sl], in_=htlo_d[:, sl])

                # per-head broadcast of src scores: g[n] = exp(-0.8 * s_h[n])
                # (bf16 hi/lo split for ~fp32 accuracy at bf16 matmul rate)
                for h in range(NH):
                    bigps = psB.tile([128, BLK], F32, tag="bigB",
                                     name=f"bc{h}")
                    vh = vrh_sb[:, h * 128:(h + 1) * 128]
                    vl = vrl_sb[:, h * 128:(h + 1) * 128]
                    for c0, c1 in ((0, 512), (512, BLK)):
                        nc.tensor.matmul(bigps[:, c0:c1], vh, hbh_sb[:, c0:c1],
                                         start=True, stop=False)
                        nc.tensor.matmul(bigps[:, c0:c1], vl, hbh_sb[:, c0:c1],
                                         start=False, stop=False)
                        nc.tensor.matmul(bigps[:, c0:c1], vh, hbl_sb[:, c0:c1],
                                         start=False, stop=True)
                    nc.scalar.activation(out=gb_sb[:, h * BLK:(h + 1) * BLK],
                                         in_=bigps[:], func=AF.Exp, scale=-0.8)

                # h_prime (4 heads side by side) + dst scores, all 48 m-tiles,
                # bf16 hi/lo split: hi@whi + lo@whi + hi@wlo ~= fp32 accuracy
                for t in range(NT):
                    hp_ps = psA.tile([128, NH * F_OUT + NH], F32,
                                     name="hp_ps", tag="hp_ps")
                    hi = hthi_sb[:, t * 128:(t + 1) * 128]
                    lo = htlo_sb[:, t * 128:(t + 1) * 128]
                    nc.tensor.matmul(hp_ps[:], hi, wvh_sb[:],
                                     start=True, stop=False)
                    nc.tensor.matmul(hp_ps[:], lo, wvh_sb[:],
                                     start=False, stop=False)
                    nc.tensor.matmul(hp_ps[:], hi, wvl_sb[:],
                                     start=False, stop=True)
                    sc = hp_ps[:, 128:132]
                    nc.scalar.activation(out=rr_sb[:, t * 4:(t + 1) * 4],
                                         in_=sc, func=AF.Exp, scale=0.8)
                    nc.scalar.activation(out=e02_sb[:, t * 4:(t + 1) * 4],
                                         in_=sc, func=AF.Exp, scale=0.2)
                    dst = hp_sb[:, t * 132:(t + 1) * 132].rearrange(
                        "p (h x) -> p h x", h=NH)[:, :, 0:32]
                    src = hp_ps[:, 0:128].rearrange("p (h x) -> p h x", h=NH)
                    nc.scalar.activation(out=dst, in_=src, func=AF.Copy)
                    if ACT_EVERY and t % ACT_EVERY == ACT_T0:
                        s4 = slice(t * 4, (t + 1) * 4)
                        # c[m] = e02*rr = e^{d[m]}; ACT path needs -c (bias)
                        # and +c in bf16 (rank-1 correction matmul rhs)
                        nc.vector.tensor_scalar(
                            out=mc_sb[:, s4], in0=e02_sb[:, s4],
                            scalar1=-1.0, scalar2=None, op0=OP.mult)
                        nc.vector.tensor_tensor(
                            out=mc_sb[:, s4], in0=mc_sb[:, s4],
                            in1=rr_sb[:, s4], op=OP.mult)
                        nc.vector.tensor_scalar(
                            out=cb_sb[:, s4], in0=mc_sb[:, s4],
                            scalar1=-1.0, scalar2=None, op0=OP.mult)

                # hot phase: two heads per pass via PE column tiling.
                for pair in range(2):
                    hA, hB = 2 * pair, 2 * pair + 1
                    # separate PSUM tiles (distinct banks) so start=True bank
                    # clears don't interact; B sits at partitions 64..96 to
                    # match its column-group position.
                    psA_o = psB.tile([33, BLK + 1], F32, tag="bigA",
                                     name=f"accA{orep}_{pair}")
                    psB_o = psB.tile([97, BLK + 1], F32, tag="bigB",
                                     name=f"accB{orep}_{pair}")
                    gA = gb_sb[:, hA * BLK:(hA + 1) * BLK]
                    gB = gb_sb[:, hB * BLK:(hB + 1) * BLK]
                    act_ts = [t for t in range(NT)
                              if ACT_EVERY and t % ACT_EVERY == ACT_T0]
                    for t in range(NT):
                        on_act = t in act_ts
                        ptA = ppool.tile([128, BLK], PDT, name="ptA", tag="ptA")
                        ptB = ppool.tile([128, BLK], PDT, name="ptB", tag="ptB")
                        for pt, g, hh in ((ptA, gA, hA), (ptB, gB, hB)):
                            c = slice(t * 4 + hh, t * 4 + hh + 1)
                            if on_act:
                                # P' - e^d = relu(e02*g - e^d); the missing
                                # rank-1 e^d term is restored via the spare
                                # PSUM column + fused bias at copy-out.
                                nc.scalar.activation(
                                    out=pt[:], in_=g, func=AF.Relu,
                                    scale=e02_sb[:, c], bias=mc_sb[:, c])
                            else:
                                nc.vector.tensor_scalar(
                                    out=pt[:], in0=g,
                                    scalar1=rr_sb[:, c], scalar2=e02_sb[:, c],
                                    op0=OP.max, op1=OP.mult)
                        lA = hp_sb[:, t * 132 + hA * 33:t * 132 + hA * 33 + 33]
                        lB = hp_sb[:, t * 132 + hB * 33:t * 132 + hB * 33 + 33]
                        st, sp = (t == 0), (t == NT - 1)
                        nc.tensor.matmul(psA_o[0:33, 0:512], lA, ptA[:, 0:512],
                                         start=st, stop=sp,
                                         tile_position=(0, 0))
                        nc.tensor.matmul(psB_o[64:97, 0:512], lB, ptB[:, 0:512],
                                         start=st, stop=sp,
                                         tile_position=(0, 64))
                        nc.tensor.matmul(psA_o[0:33, 512:BLK], lA,
                                         ptA[:, 512:BLK], start=st, stop=sp,
                                         tile_position=(0, 0))
                        nc.tensor.matmul(psB_o[64:97, 512:BLK], lB,
                                         ptB[:, 512:BLK], start=st, stop=sp,
                                         tile_position=(0, 64))
                        if on_act:
                            # rank-1 correction: accumulate hp.T @ e^d into the
                            # spare column. start=False always: the t=0 main
                            # matmul already bank-cleared has_written, so the
                            # first write lands as overwrite, later ones add.
                            cst = (t == act_ts[-1])
                            nc.tensor.matmul(
                                psA_o[0:33, BLK:BLK + 1], lA,
                                cb_sb[:, t * 4 + hA:t * 4 + hA + 1],
                                start=False, stop=cst, skip_group_check=True,
                                tile_position=(0, 0))
                            nc.tensor.matmul(
                                psB_o[64:97, BLK:BLK + 1], lB,
                                cb_sb[:, t * 4 + hB:t * 4 + hB + 1],
                                start=False, stop=cst, skip_group_check=True,
                                tile_position=(0, 64))

                    # normalize + transpose back to [n, o]
                    for h, ps_o, r0 in ((hA, psA_o, 0), (hB, psB_o, 64)):
                        accv = ps_o[r0:r0 + 33, 0:BLK]
                        oT = otp.tile([33, BLK], F32, name="oT", tag="oT")
                        if act_ts:
                            ucol = rcp.tile([33, 1], F32, name="ucol",
                                            tag="ucol")
                            nc.vector.tensor_copy(
                                ucol[:], ps_o[r0:r0 + 33, BLK:BLK + 1])
                            nc.scalar.activation(out=oT[:], in_=accv,
                                                 func=AF.Identity,
                                                 bias=ucol[:])
                        else:
                            nc.scalar.activation(out=oT[:], in_=accv,
                                                 func=AF.Copy)
                        for j in range(6):
                            tp = psC.tile([128, 33], F32, name="tp", tag="tp")
                            nc.tensor.transpose(tp[:],
                                                oT[:, j * 128:(j + 1) * 128],
                                                id_sb[:])
                            rc = rcp.tile([128, 1], F32, name="rc", tag="rc")
                            nc.vector.reciprocal(rc[:], tp[:, 32:33])
                            nc.scalar.mul(
                                outsb[:, j * 128 + h * 32:
                                      j * 128 + (h + 1) * 32],
                                tp[:, 0:32], rc[:])

                nc.sync.dma_start(
                    out=out_d[:].rearrange("(c p) o -> p c o", p=128),
                    in_=outsb[:].rearrange("p (c o) -> p c o", c=6))

    patched = _split_waits(type(nc).to_json_bytes(nc))
    nc.to_json_bytes = lambda: patched
    return nc


class _Runner:
    """Cached jitted shard_map over 8 cores (mirrors bass2jax.run_bass_via_pjrt
    but builds the jit once; outputs not donated — the kernel writes every
    output element, so fresh uninitialized result buffers are fine)."""

    def __init__(self, nc):
        import jax
        from jax.sharding import Mesh, PartitionSpec
        from jax.experimental.shard_map import shard_map
        import concourse.bass2jax as bass2jax

        bass2jax.install_neuronx_cc_hook()
        self.jax = jax
        in_names, out_names, out_avals, zeros = [], [], [], []
        part_name = nc.partition_id_tensor.name if nc.partition_id_tensor else None
        for alloc in nc.m.functions[0].allocations:
            if not isinstance(alloc, mybir.MemoryLocationSet):
                continue
            name = alloc.memorylocations[0].name
            if alloc.kind == "ExternalInput":
                if name != part_name:
                    in_names.append(name)
            elif alloc.kind == "ExternalOutput":
                out_names.append(name)
                shape = tuple(alloc.tensor_shape)
                dtype = mybir.dt.np(alloc.dtype)
                out_avals.append(jax.core.ShapedArray(shape, dtype))
                zeros.append(np.zeros((NCORES * shape[0],) + shape[1:], dtype))
        self.in_names, self.out_names, self.out_avals = in_names, out_names, out_avals
        all_names = in_names + out_names + ([part_name] if part_name else [])

        def _body(*args):
            operands = list(args)
            if part_name:
                operands.append(bass2jax.partition_id_tensor())
            outs = bass2jax._bass_exec_p.bind(
                *operands, out_avals=tuple(out_avals),
                in_names=tuple(all_names), out_names=tuple(out_names),
                lowering_input_output_aliases=(),
                sim_require_finite=True, sim_require_nnan=True, nc=nc)
            return tuple(outs)

        devices = jax.devices()[:NCORES]
        mesh = Mesh(np.asarray(devices), ("core",))
        nin = len(in_names) + len(out_names)
        self.fn = jax.jit(shard_map(
            _body, mesh=mesh, in_specs=(PartitionSpec("core"),) * nin,
            out_specs=(PartitionSpec("core"),) * len(out_names),
            check_rep=False), keep_unused=True)
        self.dev_zeros = [jax.device_put(z) for z in zeros]

    def stage(self, in_maps):
        cat = [np.concatenate([np.asarray(in_maps[c][nm])
                               for c in range(NCORES)], axis=0)
               for nm in self.in_names]
        return [self.jax.device_put(a) for a in cat]

    def run_staged(self, dev_in):
        return self.fn(*dev_in, *self.dev_zeros)

    def __call__(self, in_maps):
        outs = self.run_staged(self.stage(in_maps))
        o = np.asarray(outs[0]).reshape(NCORES, BLK, NH * F_OUT)
        return o.reshape(N, NH * F_OUT)


def _prep_inputs(h, w, a_src, a_dst, b):
    bf16 = ml_dtypes.bfloat16
    h = np.asarray(h, np.float32)
    w = np.asarray(w, np.float32)
    a_src = np.asarray(a_src, np.float32).reshape(NH, F_OUT)
    a_dst = np.asarray(a_dst, np.float32).reshape(NH, F_OUT)
    b = np.asarray(b, np.float32)

    def hilo(a):
        hi = a.astype(bf16)
        lo = (a - hi.astype(np.float32)).astype(bf16)
        return hi, lo

    hT = np.ascontiguousarray(h.T)                                # [64, N]
    ht_hi, ht_lo = hilo(hT)
    v_src = np.einsum("hfo,ho->hf", w, a_src)                     # [4, 64]
    v_dst = np.einsum("hfo,ho->hf", w, a_dst)                     # [4, 64]
    wv = np.concatenate([w.transpose(1, 0, 2).reshape(F_IN, NH * F_OUT),
                         v_dst.T], axis=1)                        # [64, 132]
    wv_hi, wv_lo = hilo(wv)
    vrep = np.repeat(v_src[:, :, None], 128, axis=2)              # [4, 64, 128]
    vrep = np.ascontiguousarray(vrep.transpose(1, 0, 2).reshape(F_IN, NH * 128))
    vr_hi, vr_lo = hilo(vrep)
    identb = np.eye(33, dtype=np.float32)
    identb[32, 0:32] = b                     # bias folds into the transpose:
    # (oT.T @ identb)[n, o] = out_un[n, o] + denom[n]*b[o]; dividing by
    # denom[n] afterwards yields softmax-output + b directly.

    common = {"hthi": ht_hi, "htlo": ht_lo, "identb": identb}
    in_maps = []
    for c in range(NCORES):
        m = dict(common)
        blk = hT[:, c * BLK:(c + 1) * BLK]
        hb_hi, hb_lo = hilo(np.ascontiguousarray(blk))
        m["pack"] = np.ascontiguousarray(np.concatenate(
            [hb_hi, hb_lo, wv_hi, wv_lo, vr_hi, vr_lo], axis=1))
        in_maps.append(m)
    return in_maps


def kernel(h, w, a_src, a_dst, b):
    if "runner" not in _cache:
        _cache["nc"] = _build_program()
        _cache["runner"] = _Runner(_cache["nc"])
    out = _cache["runner"](_prep_inputs(h, w, a_src, a_dst, b))
    return out.astype(np.float32)
